# revision 1
# baseline (speedup 1.0000x reference)
"""Trainium2 Bass kernel for causal multi-head attention with adaptive
temperature (entropy-polynomial) softmax.

Problem shape: x [2, 2048, 1024], 16 heads x 64 dims, causal.
  q/k/v = x @ W{q,k,v}.T ; sim = q k^T / 8 (causal) ;
  attn = softmax(beta * sim), beta = f(entropy(softmax(sim))) ;
  out = (attn v) @ Wo.T + bo

Sharding (8 cores): core c owns batch b = c // 4 and heads
4*(c%4) .. 4*(c%4)+3.  Each core computes its heads' q/k/v projections
(tensor-parallel over the head dim), full [n, n] score tiles for its
heads, and a partial output projection over its 256 channel dims.
Host sums the 4 partials per batch and adds bo.

Device-side per core:
  phase A : qT/kT = (Wq/Wk slice) @ x^T  (f32r, qT pre-scaled by 1/8),
            v = x @ Wv_slice.T cast to bf16
  phase B1: entropy-stat sweep over all (row-block, head):
            scores chunk -> exp (accum Z1) -> l*exp(l) (accum D)
  stats   : one batched [128, 64] chain:
            H = ln Z1 - D/Z1 ; beta = where(H>.5, max(poly(H),1), 1)
  phase B2: rescore -> exp(beta*l) (accum Z2) -> normalize (bf16) ->
            DMA-xbar transpose -> attn^T @ v accumulation (bf16)
  phase C : partial = attn_out^T.T @ Wo_slice^T  (f32r)
"""

import numpy as np

import concourse.bass as bass
import concourse.tile as tile
from concourse import bacc, mybir
from concourse.bass_utils import run_bass_kernel_spmd
from concourse.masks import make_identity

F32 = mybir.dt.float32
F32R = mybir.dt.float32r
BF16 = mybir.dt.bfloat16
I32 = mybir.dt.int32
AFT = mybir.ActivationFunctionType
ALU = mybir.AluOpType

B, N, DIM = 2, 2048, 1024
H_TOT, HD = 16, 64
N_CORES = 8
NH = 4            # heads per core
CD = NH * HD      # 256 channel dims per core
NRB = N // 128    # 16 row blocks
NU = NRB * NH     # 64 (rb, head) units
POLY = [-0.037, 0.481, -2.3, 4.917, -1.791]
MASK_VAL = -1e30
SCALE = 1.0 / 8.0  # 1/sqrt(64)

CHUNK = 512          # score-chunk width (PSUM tile free size)
USE_DMA_T = True      # transpose attn via DMA xbar instead of PE


def _scores(nc, ps, q_l, kTm, base, off, cw):
    """matmul score chunk [128, cw] at row offset `off` into psum `ps`."""
    for o2 in range(0, cw, 512):
        sw = min(512, cw - o2)
        nc.tensor.matmul(ps[:, o2:o2 + sw], q_l,
                         kTm[base:base + 64, off + o2:off + o2 + sw],
                         start=True, stop=True)


def build_kernel():
    nc = bacc.Bacc("TRN2", target_bir_lowering=False, debug=False,
                   num_devices=N_CORES)

    xT = nc.dram_tensor("xT", [DIM, N], F32, kind="ExternalInput").ap()
    wqT = nc.dram_tensor("wqT", [DIM, CD], F32, kind="ExternalInput").ap()
    wkT = nc.dram_tensor("wkT", [DIM, CD], F32, kind="ExternalInput").ap()
    wvT = nc.dram_tensor("wvT", [DIM, CD], F32, kind="ExternalInput").ap()
    woT = nc.dram_tensor("woT", [CD, DIM], F32, kind="ExternalInput").ap()
    maskin = nc.dram_tensor("maskin", [128, 128], F32, kind="ExternalInput").ap()
    partial = nc.dram_tensor("partial", [N, DIM], F32, kind="ExternalOutput").ap()

    KC = DIM // 128  # 8 contraction chunks

    with tile.TileContext(nc) as tc:
        # ---- persistent pools (allocated first = live whole kernel) ----
        with tc.tile_pool(name="const", bufs=1) as constp, \
             tc.tile_pool(name="qkv_sb", bufs=1) as qkvp, \
             tc.tile_pool(name="attn_out", bufs=1) as aop, \
             tc.tile_pool(name="wo_sb", bufs=1) as wop, \
             tc.tile_pool(name="statsall", bufs=1) as sap:

            ident = constp.tile([128, 128], BF16)
            make_identity(nc, ident[:])
            mask = constp.tile([128, 128], F32)
            nc.sync.dma_start(mask[:], maskin[:])
            ones64 = constp.tile([128, NU], F32)
            nc.vector.memset(ones64[:], 1.0)

            # persistent activations
            qT = [qkvp.tile([128, N], F32R, tag=f"qT{m}", name=f"qT{m}") for m in range(2)]
            kT = [qkvp.tile([128, N], F32R, tag=f"kT{m}", name=f"kT{m}") for m in range(2)]
            v_bf = [qkvp.tile([128, CD], BF16, tag=f"v{j}", name=f"v{j}") for j in range(NRB)]
            attT = [aop.tile([128, N], F32R, tag=f"attT{m}", name=f"attT{m}") for m in range(2)]
            woS = [wop.tile([128, DIM], F32R, tag=f"wo{m}", name=f"wo{m}") for m in range(2)]

            Z1a = sap.tile([128, NU], F32)
            D1a = sap.tile([128, NU], F32)
            Z1p = sap.tile([128, 4 * NU], F32)
            D1p = sap.tile([128, 4 * NU], F32)
            beta_all = sap.tile([128, NU], F32)

            # ---- phase A: QKV projections ----
            with tc.tile_pool(name="xw_sb", bufs=1) as xwp, \
                 tc.tile_pool(name="qkv_ps", bufs=4, space="PSUM") as qkps:
                xTs = [xwp.tile([128, N], F32R, tag=f"xT{k}", name=f"xTs{k}") for k in range(KC)]
                wq_s = [xwp.tile([128, CD], F32R, tag=f"wq{k}", name=f"wq{k}") for k in range(KC)]
                wk_s = [xwp.tile([128, CD], F32R, tag=f"wk{k}", name=f"wk{k}") for k in range(KC)]
                wv_s = [xwp.tile([128, CD], F32R, tag=f"wv{k}", name=f"wv{k}") for k in range(KC)]
                # per-k interleave so the k=0 accumulation steps can start
                # after ~0.5 MB of DMA instead of after the whole 11 MB
                for k in range(KC):
                    sl = slice(128 * k, 128 * (k + 1))
                    nc.sync.dma_start(wq_s[k][:], wqT[sl, :].bitcast(F32R))
                    nc.sync.dma_start(wk_s[k][:], wkT[sl, :].bitcast(F32R))
                    nc.sync.dma_start(wv_s[k][:], wvT[sl, :].bitcast(F32R))
                    nc.sync.dma_start(xTs[k][:], xT[sl, :].bitcast(F32R))
                for m in range(2):
                    nc.sync.dma_start(woS[m][:], woT[128 * m:128 * (m + 1), :].bitcast(F32R))

                # qT / kT: [o, r] = sum_c W[o,c] x[r,c]
                # emit per head-pair (q then k) so attention on pair 0 can
                # start while pair 1 / v are still projecting
                for m in range(2):
                    for which, wt, dest, scl in (("q", wq_s, qT, SCALE), ("k", wk_s, kT, 1.0)):
                        for nn in range(N // 512):
                            pq = qkps.tile([128, 512], F32, tag="pq")
                            for k in range(KC):
                                nc.tensor.matmul(
                                    pq[:], wt[k][:, 128 * m:128 * (m + 1)],
                                    xTs[k][:, 512 * nn:512 * (nn + 1)],
                                    start=(k == 0), stop=(k == KC - 1))
                            nc.scalar.activation(
                                dest[m][:, 512 * nn:512 * (nn + 1)], pq[:],
                                AFT.Copy, bias=0.0, scale=scl)

                # v: [j, d] = sum_c xT[c,j] wvT[c,d]
                for jt in range(NRB):
                    pv = qkps.tile([128, CD], F32, tag="pv")
                    for k in range(KC):
                        nc.tensor.matmul(
                            pv[:], xTs[k][:, 128 * jt:128 * (jt + 1)], wv_s[k][:],
                            start=(k == 0), stop=(k == KC - 1))
                    nc.any.tensor_copy(v_bf[jt][:], pv[:])

            # ---- phase B ----
            with tc.tile_pool(name="scr", bufs=8) as scrp, \
                 tc.tile_pool(name="t2p", bufs=2) as t2p, \
                 tc.tile_pool(name="t2bfp", bufs=2) as t2bfp, \
                 tc.tile_pool(name="ttp", bufs=8) as ttp, \
                 tc.tile_pool(name="stats", bufs=4) as stp, \
                 tc.tile_pool(name="ost", bufs=2) as ostp:

                # ---- B1: entropy-stat sweep ----
                # per-chunk accumulators land directly in wide [128, 4*NU]
                # tiles; one batched 3D-AP reduce replaces the per-unit ones
                nc.vector.memset(Z1p[:], 0.0)
                nc.vector.memset(D1p[:], 0.0)
                with tc.tile_pool(name="b1_ps", bufs=8, space="PSUM") as scps:
                    for rb in range(NRB):
                        W = 128 * (rb + 1)
                        chunks = [(off, min(CHUNK, W - off)) for off in range(0, W, CHUNK)]
                        nck = len(chunks)
                        for h in range(NH):
                            col = rb * NH + h
                            m, base = h // 2, 64 * (h % 2)
                            q_l = qT[m][base:base + 64, 128 * rb:128 * (rb + 1)]
                            for ci, (off, cw) in enumerate(chunks):
                                ps = scps.tile([128, CHUNK], F32, tag="ps_s")
                                _scores(nc, ps, q_l, kT[m], base, off, cw)
                                if off + cw == W:
                                    nc.vector.tensor_tensor(
                                        out=ps[:, cw - 128:cw], in0=ps[:, cw - 128:cw],
                                        in1=mask[:], op=ALU.add)
                                t1 = scrp.tile([128, CHUNK], F32, tag="t1")
                                nc.scalar.activation(
                                    t1[:, :cw], ps[:, :cw], AFT.Exp,
                                    bias=0.0, scale=1.0,
                                    accum_out=Z1p[:, 4 * col + ci:4 * col + ci + 1])
                                s2 = scrp.tile([128, CHUNK], F32, tag="s2")
                                nc.vector.scalar_tensor_tensor(
                                    out=s2[:, :cw], in0=ps[:, :cw], scalar=1.0,
                                    in1=t1[:, :cw], op0=ALU.mult, op1=ALU.mult,
                                    accum_out=D1p[:, 4 * col + ci:4 * col + ci + 1])

                # ---- batched stats: H = ln Z1 - D/Z1 ; beta ----
                nc.vector.tensor_reduce(
                    out=Z1a[:], in_=Z1p.rearrange("p (u c) -> p u c", c=4),
                    axis=mybir.AxisListType.X, op=ALU.add)
                nc.vector.tensor_reduce(
                    out=D1a[:], in_=D1p.rearrange("p (u c) -> p u c", c=4),
                    axis=mybir.AxisListType.X, op=ALU.add)
                rz = stp.tile([128, NU], F32, tag="rz")
                nc.vector.reciprocal(rz[:], Z1a[:])
                dn = stp.tile([128, NU], F32, tag="dn")
                nc.vector.tensor_mul(dn[:], D1a[:], rz[:])
                lnz = stp.tile([128, NU], F32, tag="lnz")
                nc.scalar.activation(lnz[:], Z1a[:], AFT.Ln, bias=0.0, scale=1.0)
                Hent = stp.tile([128, NU], F32, tag="Hent")
                nc.vector.tensor_sub(Hent[:], lnz[:], dn[:])
                p0 = stp.tile([128, NU], F32, tag="p0")
                nc.vector.tensor_scalar(out=p0[:], in0=Hent[:], scalar1=POLY[0],
                                        scalar2=POLY[1], op0=ALU.mult, op1=ALU.add)
                p1 = stp.tile([128, NU], F32, tag="p1")
                for c in POLY[2:]:
                    nc.vector.tensor_mul(p1[:], p0[:], Hent[:])
                    nc.vector.tensor_scalar_add(p0[:], p1[:], c)
                nc.vector.tensor_scalar_max(p1[:], p0[:], 1.0)
                mk = stp.tile([128, NU], I32, tag="mk")
                nc.vector.tensor_scalar(out=mk[:], in0=Hent[:], scalar1=0.5,
                                        scalar2=None, op0=ALU.is_gt)
                nc.vector.tensor_copy(beta_all[:], ones64[:])
                nc.vector.copy_predicated(beta_all[:], mk[:], p1[:])

                # ---- B2: weighted softmax + attn @ v ----
                with tc.tile_pool(name="b2_ps", bufs=4, space="PSUM") as scps2, \
                     tc.tile_pool(name="tp_ps", bufs=3, space="PSUM") as tpps, \
                     tc.tile_pool(name="av_ps", bufs=1, space="PSUM") as avps_pool:
                    for rb in range(NRB):
                        W = 128 * (rb + 1)
                        njt = rb + 1
                        chunks = [(off, min(CHUNK, W - off)) for off in range(0, W, CHUNK)]
                        nck = len(chunks)
                        avp = None
                        for h in range(NH):
                            col = rb * NH + h
                            m, base = h // 2, 64 * (h % 2)
                            q_l = qT[m][base:base + 64, 128 * rb:128 * (rb + 1)]
                            z2c = stp.tile([128, 4], F32, tag="z2c")
                            z2s = stp.tile([128, 1], F32, tag="z2s")
                            t2 = t2p.tile([128, N], F32, tag="t2")
                            for ci, (off, cw) in enumerate(chunks):
                                ps2 = scps2.tile([128, CHUNK], F32, tag="ps_s")
                                _scores(nc, ps2, q_l, kT[m], base, off, cw)
                                if off + cw == W:
                                    nc.vector.tensor_tensor(
                                        out=ps2[:, cw - 128:cw], in0=ps2[:, cw - 128:cw],
                                        in1=mask[:], op=ALU.add)
                                nc.scalar.activation(
                                    t2[:, off:off + cw], ps2[:, :cw], AFT.Exp,
                                    bias=0.0, scale=beta_all[:, col:col + 1],
                                    accum_out=(z2c[:, ci:ci + 1] if nck > 1 else z2s[:]))
                            if nck > 1:
                                nc.vector.tensor_reduce(out=z2s[:], in_=z2c[:, :nck],
                                                        axis=mybir.AxisListType.X,
                                                        op=ALU.add)
                            rz2 = stp.tile([128, 1], F32, tag="rz2")
                            nc.vector.reciprocal(rz2[:], z2s[:])
                            t2bf = t2bfp.tile([128, N], BF16, tag="t2bf")
                            nc.vector.tensor_scalar_mul(t2bf[:, :W], t2[:, :W], rz2[:])

                            # transpose + av
                            if h % 2 == 0:
                                avp = avps_pool.tile([128, 128], F32, tag="avp")
                            for g in range(0, njt, 4):
                                gn = min(4, njt - g)
                                tp = tpps.tile([128, 512], BF16, tag="tp")
                                for kk in range(gn):
                                    jt = g + kk
                                    nc.tensor.transpose(
                                        tp[:, 128 * kk:128 * (kk + 1)],
                                        t2bf[:, 128 * jt:128 * (jt + 1)], ident[:])
                                tts = ttp.tile([128, 512], BF16, tag="tts")
                                nc.any.tensor_copy(tts[:, :128 * gn], tp[:, :128 * gn])
                                for kk in range(gn):
                                    jt = g + kk
                                    nc.tensor.matmul(
                                        avp[base:base + 64, :],
                                        v_bf[jt][:, 64 * h:64 * (h + 1)],
                                        tts[:, 128 * kk:128 * (kk + 1)],
                                        start=(jt == 0), stop=(jt == njt - 1),
                                        tile_position=(0, base))
                            if h % 2 == 1:
                                nc.any.tensor_copy(attT[m][:, 128 * rb:128 * (rb + 1)], avp[:])
                # ---- phase C: output projection ----
                with tc.tile_pool(name="pj_ps", bufs=2, space="PSUM") as pjps:
                    for rb in range(NRB):
                        for nn in range(2):
                            pp = pjps.tile([128, 512], F32, tag="pp")
                            for m in range(2):
                                nc.tensor.matmul(
                                    pp[:], attT[m][:, 128 * rb:128 * (rb + 1)],
                                    woS[m][:, 512 * nn:512 * (nn + 1)],
                                    start=(m == 0), stop=(m == 1))
                            ost = ostp.tile([128, 512], F32, tag="ost")
                            nc.any.tensor_copy(ost[:], pp[:])
                            nc.sync.dma_start(
                                partial[128 * rb:128 * (rb + 1), 512 * nn:512 * (nn + 1)],
                                ost[:])

    nc.compile()
    return nc


_NC_CACHE = None
_LAST_IN_MAPS = None


def kernel(x, Wq, Wk, Wv, Wo, bo):
    global _NC_CACHE, _LAST_IN_MAPS
    x = np.asarray(x, dtype=np.float32)
    Wq = np.asarray(Wq, dtype=np.float32)
    Wk = np.asarray(Wk, dtype=np.float32)
    Wv = np.asarray(Wv, dtype=np.float32)
    Wo = np.asarray(Wo, dtype=np.float32)
    bo = np.asarray(bo, dtype=np.float32)

    if _NC_CACHE is None:
        _NC_CACHE = build_kernel()
    nc = _NC_CACHE

    mask_h = np.where(np.arange(128)[None, :] > np.arange(128)[:, None],
                      np.float32(MASK_VAL), np.float32(0.0)).astype(np.float32)
    woT_full = np.ascontiguousarray(Wo.T)  # [c, o]

    in_maps = []
    for c in range(N_CORES):
        b = c // 4
        s0 = CD * (c % 4)
        sl = slice(s0, s0 + CD)
        in_maps.append({
            "xT": np.ascontiguousarray(x[b].T),
            "wqT": np.ascontiguousarray(Wq[sl, :].T),
            "wkT": np.ascontiguousarray(Wk[sl, :].T),
            "wvT": np.ascontiguousarray(Wv[sl, :].T),
            "woT": np.ascontiguousarray(woT_full[sl, :]),
            "maskin": mask_h,
        })

    _LAST_IN_MAPS = in_maps
    res = run_bass_kernel_spmd(nc, in_maps, core_ids=list(range(N_CORES)))

    out = np.zeros((B, N, DIM), dtype=np.float32)
    for c in range(N_CORES):
        out[c // 4] += res.results[c]["partial"]
    out += bo[None, None, :]
    return out



# revision 18
# speedup vs baseline: 1.0611x; 1.0611x over previous
"""Trainium2 Bass kernel for causal multi-head attention with adaptive
temperature (entropy-polynomial) softmax.

Problem shape: x [2, 2048, 1024], 16 heads x 64 dims, causal.
  q/k/v = x @ W{q,k,v}.T ; sim = q k^T / 8 (causal) ;
  attn = softmax(beta * sim), beta = f(entropy(softmax(sim))) ;
  out = (attn v) @ Wo.T + bo

Sharding (8 cores): core c owns batch b = c // 4 and heads
4*(c%4) .. 4*(c%4)+3 (tensor-parallel over the head dim).  Host sums
the 4 partials per batch and adds bo.

Device-side per core (all matmuls bf16, 1 cycle/row):
  phase A : qT/kT = (Wq/8, Wk slice) @ x^T -> bf16, v row-layout bf16
            with a ones column appended per head (Z2 trick)
  phase B1: row-layout score chunks [128, <=1024] -> exp (accum Z1)
            -> l*exp(l) (accum D); causal mask applied by an extra
            accumulating PE matmul (identity @ mask)
  stats   : batched: H = ln Z1 - D/Z1 ; beta = where(H>.5, max(poly,1), 1)
            beta transposed (PE) -> rows -> DMA-broadcast -> q2T = qT*beta
  phase B2: TRANSPOSED scores sT[j, i] = k_l^T q2T (no PE transposes!)
            -> exp -> bf16 expT tiles; AV accumulates v_aug^T @ expT
            over j-blocks into [65, 512] psum per i-chunk; the extra
            ones row yields Z2; normalize folded into the psum->SBUF
            copy via DMA-broadcast 1/Z2 tiles
  phase C : partial = attT^T @ Wo_slice (bf16)
"""

import numpy as np
import ml_dtypes

import concourse.bass as bass
import concourse.tile as tile
from concourse import bacc, mybir
from concourse.bass_utils import run_bass_kernel_spmd
from concourse.masks import make_identity

F32 = mybir.dt.float32
BF16 = mybir.dt.bfloat16
I32 = mybir.dt.int32
AFT = mybir.ActivationFunctionType
ALU = mybir.AluOpType

B, N, DIM = 2, 2048, 1024
H_TOT, HD = 16, 64
N_CORES = 8
NH = 4            # heads per core
CD = NH * HD      # 256 channel dims per core
NRB = N // 128    # 16 row blocks
NU = NH * NRB     # 64 (h, rb) units, h-major: u = h*16 + rb
POLY = [-0.037, 0.481, -2.3, 4.917, -1.791]
MASK_VAL = -1e30

KC = DIM // 128   # 8 contraction chunks
B1CHUNK = 1024    # B1 score-chunk width (2 psum banks)
B2CHUNK = 1024    # B2 transposed-score chunk width


def build_kernel():
    nc = bacc.Bacc("TRN2", target_bir_lowering=False, debug=False,
                   num_devices=N_CORES)

    xT = nc.dram_tensor("xT", [DIM, N], BF16, kind="ExternalInput").ap()
    wqT = nc.dram_tensor("wqT", [DIM, CD], BF16, kind="ExternalInput").ap()
    wkT = nc.dram_tensor("wkT", [DIM, CD], BF16, kind="ExternalInput").ap()
    wvT = nc.dram_tensor("wvT", [DIM, CD], BF16, kind="ExternalInput").ap()
    woT = nc.dram_tensor("woT", [CD, DIM], BF16, kind="ExternalInput").ap()
    maskin = nc.dram_tensor("maskin", [128, 128], BF16, kind="ExternalInput").ap()
    maskTin = nc.dram_tensor("maskTin", [128, 128], BF16, kind="ExternalInput").ap()
    partial = nc.dram_tensor("partial", [N, DIM], F32, kind="ExternalOutput").ap()
    # DRAM scratch for partition-broadcast bounces (SBUF-src bcast illegal)
    beta_dram = nc.dram_tensor("beta_scr", [NU, 128], BF16, kind="Internal").ap()
    z2_dram = nc.dram_tensor("z2_scr", [NH, N], F32, kind="Internal").ap()

    with tile.TileContext(nc) as tc:
        with tc.tile_pool(name="const", bufs=1) as constp, \
             tc.tile_pool(name="qkv_sb", bufs=1) as qkvp, \
             tc.tile_pool(name="wo_sb", bufs=1) as wop, \
             tc.tile_pool(name="att_sb", bufs=1) as attp, \
             tc.tile_pool(name="statsall", bufs=1) as sap, \
             tc.tile_pool(name="z2_sb", bufs=1) as z2p:

            ident = constp.tile([128, 128], BF16)
            make_identity(nc, ident[:])
            mask_bf = constp.tile([128, 128], BF16)
            nc.sync.dma_start(mask_bf[:], maskin[:])
            maskT_bf = constp.tile([128, 128], BF16)
            nc.sync.dma_start(maskT_bf[:], maskTin[:])
            ones64 = constp.tile([128, NU], F32)
            nc.vector.memset(ones64[:], 1.0)

            # persistent activations
            qT = [qkvp.tile([128, N], BF16, tag=f"qT{m}", name=f"qT{m}")
                  for m in range(2)]
            kT = [qkvp.tile([128, N], BF16, tag=f"kT{m}", name=f"kT{m}")
                  for m in range(2)]
            q2T = [qkvp.tile([128, N], BF16, tag=f"q2T{m}", name=f"q2T{m}")
                   for m in range(2)]
            # v_aug: per jt block of 260 cols; per head a 65-col slot
            # [v(64), ones] -> AV out rows 0..64 (z2 row at 64)
            v_aug = qkvp.tile([128, NRB * 260], BF16, name="v_aug")
            woS = [wop.tile([64, DIM], BF16, tag=f"wo{h}", name=f"wo{h}")
                   for h in range(NH)]
            attTbf = [attp.tile([64, N], BF16, tag=f"attT{h}", name=f"attT{h}")
                      for h in range(NH)]

            Z1p = sap.tile([128, 2 * NU], F32)
            D1p = sap.tile([128, 2 * NU], F32)
            beta_all = sap.tile([128, NU], F32)
            beta_bf = sap.tile([128, NU], BF16)
            betaT = sap.tile([128, 128], BF16)       # rows 0:64 used
            beta_bc = [sap.tile([128, N], BF16, tag=f"bbc{m}", name=f"bbc{m}")
                       for m in range(2)]
            z2st = [z2p.tile([128, N], F32, tag=f"z2st{h}", name=f"z2st{h}")
                    for h in range(NH)]
            rz2_bc = [z2p.tile([64, N], F32, tag=f"rzbc{h}", name=f"rzbc{h}")
                      for h in range(NH)]

            # ones everywhere; v copies below overwrite the v-slots, leaving
            # 1.0 at the per-head ones-columns (even h: 65h+64; odd h: 65h)
            nc.vector.memset(v_aug[:], 1.0)

            # ---- phase A: QKV projections (bf16) ----
            with tc.tile_pool(name="xw_sb", bufs=1) as xwp, \
                 tc.tile_pool(name="qk_ps", bufs=2, space="PSUM") as qkps, \
                 tc.tile_pool(name="v_ps", bufs=2, space="PSUM") as vps:
                xTs = [xwp.tile([128, N], BF16, tag=f"xT{k}", name=f"xTs{k}")
                       for k in range(KC)]
                wq_s = [xwp.tile([128, CD], BF16, tag=f"wq{k}", name=f"wq{k}")
                        for k in range(KC)]
                wk_s = [xwp.tile([128, CD], BF16, tag=f"wk{k}", name=f"wk{k}")
                        for k in range(KC)]
                wv_s = [xwp.tile([128, CD], BF16, tag=f"wv{k}", name=f"wv{k}")
                        for k in range(KC)]
                for k in range(KC):
                    sl = slice(128 * k, 128 * (k + 1))
                    nc.sync.dma_start(wq_s[k][:], wqT[sl, :])
                    nc.sync.dma_start(wk_s[k][:], wkT[sl, :])
                    nc.sync.dma_start(wv_s[k][:], wvT[sl, :])
                    nc.sync.dma_start(xTs[k][:], xT[sl, :])
                for h in range(NH):
                    nc.sync.dma_start(woS[h][:], woT[64 * h:64 * (h + 1), :])

                # qT / kT: [o, i] = sum_c W[o,c] x[i,c]; emit m=0 first so B1
                # head 0 can start while m=1 / v still project
                for m in range(2):
                    for wt, dest in ((wq_s, qT), (wk_s, kT)):
                        for nn in range(N // 512):
                            pq = qkps.tile([128, 512], F32, tag="pq")
                            for k in range(KC):
                                nc.tensor.matmul(
                                    pq[:], wt[k][:, 128 * m:128 * (m + 1)],
                                    xTs[k][:, 512 * nn:512 * (nn + 1)],
                                    start=(k == 0), stop=(k == KC - 1))
                            nc.any.tensor_copy(
                                dest[m][:, 512 * nn:512 * (nn + 1)], pq[:])

                # v row-layout: [j, d] = sum_c xT[c,j] wvT[c,d]
                for jt in range(NRB):
                    pv = vps.tile([128, CD], F32, tag="pv")
                    for k in range(KC):
                        nc.tensor.matmul(
                            pv[:], xTs[k][:, 128 * jt:128 * (jt + 1)], wv_s[k][:],
                            start=(k == 0), stop=(k == KC - 1))
                    # v_aug per-jt layout: 4x [v_h(64), 1]
                    va = v_aug[:, 260 * jt:260 * (jt + 1)].rearrange(
                        "p (g c) -> p g c", c=65)
                    pv4 = pv[:].rearrange("p (g c) -> p g c", c=64)
                    nc.any.tensor_copy(va[:, :, 0:64], pv4[:])

            # ---- phase B1: entropy-stat sweep (row layout) ----
            nc.vector.memset(Z1p[:], 0.0)
            nc.vector.memset(D1p[:], 0.0)
            with tc.tile_pool(name="b1_ps", bufs=2, space="PSUM") as b1ps, \
                 tc.tile_pool(name="t1p", bufs=3) as t1p, \
                 tc.tile_pool(name="s2p", bufs=2) as s2p:
                for h in range(NH):
                    m, base = h // 2, 64 * (h % 2)
                    for rb in range(NRB):
                        u = h * NRB + rb
                        W = 128 * (rb + 1)
                        q_l = qT[m][base:base + 64, 128 * rb:128 * (rb + 1)]
                        for c, c0 in enumerate(range(0, W, B1CHUNK)):
                            cw = min(B1CHUNK, W - c0)
                            ps = b1ps.tile([128, B1CHUNK], F32, tag="b1")
                            d0 = W - 128 - c0  # diag offset if in this chunk
                            has_diag = 0 <= d0 < cw
                            for o in range(0, cw, 512):
                                sw = min(512, cw - o)
                                dm = has_diag and o <= d0 < o + sw
                                nc.tensor.matmul(
                                    ps[:, o:o + sw], q_l,
                                    kT[m][base:base + 64, c0 + o:c0 + o + sw],
                                    start=True, stop=not dm)
                            if has_diag:
                                nc.tensor.matmul(
                                    ps[:, d0:d0 + 128], ident[:], mask_bf[:],
                                    start=False, stop=True)
                            t1 = t1p.tile([128, B1CHUNK], F32, tag="t1")
                            nc.scalar.activation(
                                t1[:, :cw], ps[:, :cw], AFT.Exp,
                                bias=0.0, scale=1.0,
                                accum_out=Z1p[:, 2 * u + c:2 * u + c + 1])
                            s2 = s2p.tile([128, B1CHUNK], F32, tag="s2")
                            nc.vector.scalar_tensor_tensor(
                                out=s2[:, :cw], in0=ps[:, :cw], scalar=1.0,
                                in1=t1[:, :cw], op0=ALU.mult, op1=ALU.mult,
                                accum_out=D1p[:, 2 * u + c:2 * u + c + 1])

                # ---- batched stats: H = ln Z1 - D/Z1 ; beta ----
                with tc.tile_pool(name="stp", bufs=1) as stp, \
                     tc.tile_pool(name="bt_ps", bufs=1, space="PSUM") as btps:
                    Z1a = stp.tile([128, NU], F32, tag="Z1a")
                    D1a = stp.tile([128, NU], F32, tag="D1a")
                    nc.vector.tensor_reduce(
                        out=Z1a[:], in_=Z1p.rearrange("p (u c) -> p u c", c=2),
                        axis=mybir.AxisListType.X, op=ALU.add)
                    nc.vector.tensor_reduce(
                        out=D1a[:], in_=D1p.rearrange("p (u c) -> p u c", c=2),
                        axis=mybir.AxisListType.X, op=ALU.add)
                    rz = stp.tile([128, NU], F32, tag="rz")
                    nc.vector.reciprocal(rz[:], Z1a[:])
                    dn = stp.tile([128, NU], F32, tag="dn")
                    nc.vector.tensor_mul(dn[:], D1a[:], rz[:])
                    lnz = stp.tile([128, NU], F32, tag="lnz")
                    nc.scalar.activation(lnz[:], Z1a[:], AFT.Ln,
                                         bias=0.0, scale=1.0)
                    Hent = stp.tile([128, NU], F32, tag="Hent")
                    nc.vector.tensor_sub(Hent[:], lnz[:], dn[:])
                    p0 = stp.tile([128, NU], F32, tag="p0")
                    nc.vector.tensor_scalar(
                        out=p0[:], in0=Hent[:], scalar1=POLY[0],
                        scalar2=POLY[1], op0=ALU.mult, op1=ALU.add)
                    p1 = stp.tile([128, NU], F32, tag="p1")
                    for cc in POLY[2:]:
                        nc.vector.tensor_mul(p1[:], p0[:], Hent[:])
                        nc.vector.tensor_scalar_add(p0[:], p1[:], cc)
                    nc.vector.tensor_scalar_max(p1[:], p0[:], 1.0)
                    mk = stp.tile([128, NU], I32, tag="mk")
                    nc.vector.tensor_scalar(
                        out=mk[:], in0=Hent[:], scalar1=0.5, scalar2=None,
                        op0=ALU.is_gt)
                    nc.vector.tensor_copy(beta_all[:], ones64[:])
                    nc.vector.copy_predicated(beta_all[:], mk[:], p1[:])
                    nc.vector.tensor_copy(beta_bf[:], beta_all[:])

                    # beta -> betaT [u, i-in-rb] -> per-head rows -> bcast
                    btp = btps.tile([128, 128], BF16, tag="btp")
                    nc.tensor.transpose(btp[0:NU, :], beta_bf[:], ident[:])
                    nc.any.tensor_copy(betaT[0:NU, :], btp[0:NU, :])
                    nc.sync.dma_start(beta_dram[:, :], betaT[0:NU, 0:128])
                    for m in range(2):
                        for half in range(2):
                            u0 = 16 * (2 * m + half)
                            nc.sync.dma_start(
                                beta_bc[m][64 * half:64 * half + 64, :]
                                .rearrange("p (a b) -> p a b", a=16),
                                beta_dram[u0:u0 + 16, :].unsqueeze(0)
                                .to_broadcast((64, 16, 128)))
                        nc.vector.tensor_mul(q2T[m][:], qT[m][:], beta_bc[m][:])

            # ---- phase B2: transposed scores + exp + AV (+Z2) ----
            with tc.tile_pool(name="sc_ps", bufs=2, space="PSUM") as scps, \
                 tc.tile_pool(name="av_ps", bufs=1, space="PSUM") as avps, \
                 tc.tile_pool(name="expp", bufs=3) as expp:
                for h in range(NH):
                    m, base = h // 2, 64 * (h % 2)
                    avp = [avps.tile([128, 512], F32, tag=f"av{ci}",
                                     name=f"avp{h}_{ci}")
                           for ci in range(4)]
                    for jt in range(NRB):
                        i0 = 128 * jt
                        Wi = N - i0
                        eT = expp.tile([128, N], BF16, tag="eT")
                        k_l = kT[m][base:base + 64, i0:i0 + 128]
                        for c0 in range(i0, N, B2CHUNK):
                            cw = min(B2CHUNK, N - c0)
                            ps = scps.tile([128, B2CHUNK], F32, tag="sc")
                            first = (c0 == i0)
                            for o in range(0, cw, 512):
                                sw = min(512, cw - o)
                                dm = first and o == 0
                                nc.tensor.matmul(
                                    ps[:, o:o + sw], k_l,
                                    q2T[m][base:base + 64, c0 + o:c0 + o + sw],
                                    start=True, stop=not dm)
                            if first:
                                nc.tensor.matmul(
                                    ps[:, 0:128], ident[:], maskT_bf[:],
                                    start=False, stop=True)
                            nc.scalar.activation(
                                eT[:, c0 - i0:c0 - i0 + cw], ps[:, :cw],
                                AFT.Exp, bias=0.0, scale=1.0)
                        for ci in range(jt // 4, 4):
                            a = max(512 * ci, i0)
                            b2 = 512 * (ci + 1)
                            nc.tensor.matmul(
                                avp[ci][0:65, a - 512 * ci:512],
                                v_aug[:, 260 * jt + 65 * h:260 * jt + 65 * h + 65],
                                eT[:, a - i0:b2 - i0],
                                start=(jt == 0), stop=(jt == 4 * ci + 3))
                    # per-ci: 1/Z2 -> broadcast -> normalized bf16 attT
                    for ci in range(4):
                        cs = slice(512 * ci, 512 * (ci + 1))
                        nc.vector.reciprocal(
                            z2st[h][64:65, cs], avp[ci][64:65, :])
                        nc.sync.dma_start(z2_dram[h:h + 1, cs],
                                          z2st[h][64:65, cs])
                        nc.sync.dma_start(
                            rz2_bc[h][:, cs],
                            z2_dram[h:h + 1, cs].to_broadcast((64, 512)))
                        nc.vector.tensor_mul(
                            attTbf[h][:, cs],
                            avp[ci][0:64, :],
                            rz2_bc[h][:, cs])

            # ---- phase C: output projection ----
            with tc.tile_pool(name="pj_ps", bufs=2, space="PSUM") as pjps, \
                 tc.tile_pool(name="ost", bufs=2) as ostp:
                for rb in range(NRB):
                    for nn in range(2):
                        pp = pjps.tile([128, 512], F32, tag="pp")
                        for h in range(NH):
                            nc.tensor.matmul(
                                pp[:], attTbf[h][:, 128 * rb:128 * (rb + 1)],
                                woS[h][:, 512 * nn:512 * (nn + 1)],
                                start=(h == 0), stop=(h == NH - 1))
                        ost = ostp.tile([128, 512], F32, tag="ost")
                        nc.any.tensor_copy(ost[:], pp[:])
                        nc.sync.dma_start(
                            partial[128 * rb:128 * (rb + 1),
                                    512 * nn:512 * (nn + 1)],
                            ost[:])

    nc.compile()
    return nc


_NC_CACHE = None
_LAST_IN_MAPS = None


def kernel(x, Wq, Wk, Wv, Wo, bo):
    global _NC_CACHE, _LAST_IN_MAPS
    x = np.asarray(x, dtype=np.float32)
    Wq = np.asarray(Wq, dtype=np.float32)
    Wk = np.asarray(Wk, dtype=np.float32)
    Wv = np.asarray(Wv, dtype=np.float32)
    Wo = np.asarray(Wo, dtype=np.float32)
    bo = np.asarray(bo, dtype=np.float32)

    if _NC_CACHE is None:
        _NC_CACHE = build_kernel()
    nc = _NC_CACHE

    bf16 = ml_dtypes.bfloat16
    mask_h = np.where(np.arange(128)[None, :] > np.arange(128)[:, None],
                      np.float32(MASK_VAL), np.float32(0.0)).astype(bf16)
    maskT_h = np.ascontiguousarray(mask_h.T)
    woT_full = np.ascontiguousarray(Wo.T)          # [c, o]
    Wq_s = Wq * np.float32(1.0 / 8.0)              # fold 1/sqrt(hd)

    in_maps = []
    for c in range(N_CORES):
        b = c // 4
        s0 = CD * (c % 4)
        sl = slice(s0, s0 + CD)
        in_maps.append({
            "xT": np.ascontiguousarray(x[b].T).astype(bf16),
            "wqT": np.ascontiguousarray(Wq_s[sl, :].T).astype(bf16),
            "wkT": np.ascontiguousarray(Wk[sl, :].T).astype(bf16),
            "wvT": np.ascontiguousarray(Wv[sl, :].T).astype(bf16),
            "woT": np.ascontiguousarray(woT_full[sl, :]).astype(bf16),
            "maskin": mask_h,
            "maskTin": maskT_h,
        })

    _LAST_IN_MAPS = in_maps
    res = run_bass_kernel_spmd(nc, in_maps, core_ids=list(range(N_CORES)))

    out = np.zeros((B, N, DIM), dtype=np.float32)
    for c in range(N_CORES):
        out[c // 4] += res.results[c]["partial"]
    out += bo[None, None, :]
    return out
